# revision 16
# baseline (speedup 1.0000x reference)
"""CapsuleLayer (dynamic routing) Trainium2 Bass kernel.

Problem: x [64, 2048, 16], W [32, 2048, 32, 16] ->
  hat = einsum('bik,jidk->bijd', x, W); 3 routing iterations -> v [64, 32, 32].

Strategy (8 NeuronCores, In=2048 sharded 256/core; W never replicated):
  * hat is NEVER materialized. Three matmul families, all dense K=128:
      pass0:  s0 = (1/Nc) * sum_i hat  via big-K contraction over (i,k)
      (A):    agreement logits b += sum_d hat*v computed as
              G = (W . v) on PE (v folded into W), then DVE mult+reduce vs x
      (B):    s = sum_i c*hat computed as xc = c*x (DVE), DMA-transpose to
              (k,i)-partition layout, PE contraction vs W
  * s is AllReduced across cores ([128,8,64] fp32, 3x); squash computed
    redundantly on every core.
  * bf16 operands, fp32 PSUM accumulation / logits / s / v.

Layout conventions (per core, i_loc in [0,256)):
  j-map:   r = j%4, c2 = (j//4)%2, ga = j//8, jj = 4*(j//8)+j%4, slot = j//4
  ik-major ((A) path):      ik = i_loc*16 + k
  km-major ((B)/pass0):     km = k*256 + i_loc; K-tile t2 = km//128
  sT/vT canonical:          sT[32*(j%4)+d, j//4, b]
  logits:                   [(c2,b) partition, ga, r, i_loc]
"""
import sys

if "/opt/trn_rl_repo" not in sys.path:
    sys.path.insert(0, "/opt/trn_rl_repo")

from contextlib import ExitStack

import ml_dtypes
import numpy as np

import concourse.bass as bass
import concourse.mybir as mybir
import concourse.tile as tile
from concourse import bacc
from concourse.bass_utils import run_bass_kernel_spmd

B, In, Din, Nc, Dc = 64, 2048, 16, 32, 32
NCORES = 8
IL = In // NCORES  # 256
EPS = 1e-7
FP32 = mybir.dt.float32
BF16 = mybir.dt.bfloat16

_KM_K = np.arange(4096) // 256   # km-major: k index
_KM_I = np.arange(4096) % 256    # km-major: i_loc index
_IK_I = np.arange(4096) // 16    # ik-major: i_loc index
_IK_K = np.arange(4096) % 16     # ik-major: k index


def _host_prep_core(x, W, core):
    """Build per-core input arrays. x, W are the full fp32 inputs."""
    i0 = core * IL
    Wc = np.ascontiguousarray(W[:, i0:i0 + IL])        # [Nc, IL, Dc, Din]
    xc = np.ascontiguousarray(x[:, i0:i0 + IL])        # [B, IL, Din]

    km = np.arange(128)[:, None] + 128 * np.arange(32)[None, :]   # [128, 32]
    ikm, kkm = _KM_I[km], _KM_K[km]

    # Wt [128, 32, 1024] bf16: Wt[p, t2, 32j+d] = W[j, i(km), d, k(km)]
    Wt = np.empty((128, 32, 1024), np.float32)
    for j in range(Nc):
        Wt[:, :, 32 * j:32 * j + 32] = Wc[j][ikm, :, kkm]
    # Wd [128, 8, 4096] bf16: Wd[32*(j%4)+d, j//4, ik] = W[j, i_ik, d, k_ik]
    Wd = np.empty((128, 8, 4096), np.float32)
    for j in range(Nc):
        Wd[32 * (j % 4):32 * (j % 4) + 32, j // 4, :] = Wc[j][_IK_I, :, _IK_K].T
    # xT [128, 32, 64] bf16 (pre-scaled 1/Nc): xT[p, t2, b] = x[b, i(km), k(km)]/Nc
    xT = (xc[:, ikm, kkm] / Nc).transpose(1, 2, 0)
    # x2a [(c2,b), ik] bf16 ; x2k [(c2,b), km] bf16 (same data both halves)
    xa = xc[:, _IK_I, _IK_K]                           # [B, 4096]
    xk = xc[:, _KM_I, _KM_K]                           # [B, 4096]
    x2a = np.concatenate([xa, xa], axis=0)             # [128, 4096]
    x2k = np.concatenate([xk, xk], axis=0)

    bf = ml_dtypes.bfloat16
    return {
        "Wt": np.ascontiguousarray(Wt.astype(bf)),
        "Wd": np.ascontiguousarray(Wd.astype(bf)),
        "xT": np.ascontiguousarray(xT.astype(bf)),
        "x2a": np.ascontiguousarray(x2a.astype(bf)),
        "x2k": np.ascontiguousarray(x2k.astype(bf)),
        "consts": _host_consts(),
    }


def _host_consts():
    """[128, 164] fp32: identity (128), ones4 blockdiag (4), ones-rep (32)."""
    ident = np.eye(128, dtype=np.float32)
    ones4 = np.zeros((128, 4), np.float32)
    for q in range(4):
        ones4[32 * q:32 * q + 32, q] = 1.0
    onesrep = np.zeros((128, 32), np.float32)   # used as [4, 128] slice: rows 0-3
    for q in range(4):
        onesrep[q, :] = 0.0
    # ones-rep stationary: [4, 128] with onesrep[q, p] = 1 if p//32 == q
    rep = np.zeros((128, 32), np.float32)
    for p in range(128):
        rep[p % 4 if False else 0, 0] = 0.0
    # store rep as its own block: rep_block [4 rows used, 128 cols] -> pack in [128, 32]? ugly.
    # Simpler: pack all consts in one [128, 164+128] array:
    #   cols [0:128]   identity
    #   cols [128:132] ones4 blockdiag (for partition-sum over d within strip)
    #   cols [132:164] rep stationary transposed: repT[p, q*?]  -- see below
    out = np.zeros((128, 292), np.float32)
    out[:, 0:128] = ident
    out[:, 128:132] = ones4
    # scale-replicate stationary lhsT [K=4, M=128]: lhsT[q, p] = 1 if p//32==q.
    # Stored transposed-friendly: we need it as an AP [4 part, 128 free].
    repM = np.zeros((128, 128), np.float32)
    for p in range(128):
        repM[p // 32, p] = 1.0  # only rows 0-3 nonzero
    out[:, 132:260] = repM
    return np.ascontiguousarray(out)


def build_program():
    """Build the SPMD Bass/Tile program. Returns (nc, names)."""
    nc = bacc.Bacc("TRN2", target_bir_lowering=False, debug=False,
                   num_devices=NCORES)

    d_Wt = nc.dram_tensor("Wt", [128, 32, 1024], BF16, kind="ExternalInput").ap()
    d_Wd = nc.dram_tensor("Wd", [128, 8, 4096], BF16, kind="ExternalInput").ap()
    d_xT = nc.dram_tensor("xT", [128, 32, 64], BF16, kind="ExternalInput").ap()
    d_x2a = nc.dram_tensor("x2a", [128, 4096], BF16, kind="ExternalInput").ap()
    d_x2k = nc.dram_tensor("x2k", [128, 4096], BF16, kind="ExternalInput").ap()
    d_cst = nc.dram_tensor("consts", [128, 292], FP32, kind="ExternalInput").ap()
    d_out = nc.dram_tensor("out", [128, 8, 64], FP32, kind="ExternalOutput").ap()

    cc_in = nc.dram_tensor("cc_in", [128, 8, 64], FP32).ap()
    cc_out = nc.dram_tensor("cc_out", [128, 8, 64], FP32, addr_space="Shared").ap()
    core_ids = list(range(NCORES))

    with tile.TileContext(nc) as tc, ExitStack() as ctx:
        ep = ctx.enter_context
        # ------------------------------------------------ pools
        p_const = ep(tc.tile_pool(name="const", bufs=1))
        p_wstream = ep(tc.tile_pool(name="wstream", bufs=3))
        p_wm = ep(tc.tile_pool(name="wm", bufs=2))
        p_small = ep(tc.tile_pool(name="small", bufs=1))
        p_gevac = ep(tc.tile_pool(name="gevac", bufs=3))
        p_prod = ep(tc.tile_pool(name="prod", bufs=3))
        p_red = ep(tc.tile_pool(name="red", bufs=3))
        p_xc = ep(tc.tile_pool(name="xc", bufs=2))
        p_xcT = ep(tc.tile_pool(name="xcT", bufs=4))
        # Single PSUM pool, one shared tag: slot = 4 banks, 2 slots = all 8.
        p_ps_g = ep(tc.tile_pool(name="ps_g", bufs=2, space="PSUM"))

        # ------------------------------------------------ resident tiles
        cst = p_const.tile([128, 292], FP32, tag="cst")
        nc.sync.dma_start(cst[:], d_cst)
        ident = cst[:, 0:128]
        ones4 = cst[:, 128:132]          # [128, 4]: blockdiag over d-strips
        repM = cst[0:4, 132:260]         # [4, 128]: scale replicate stationary

        xT = p_const.tile([128, 32, 64], BF16, tag="xT")
        x2a = p_const.tile([128, 4096], BF16, tag="x2a")
        x2k = p_const.tile([128, 4096], BF16, tag="x2k")
        nc.sync.dma_start(xT[:], d_xT)
        nc.sync.dma_start(x2a[:], d_x2a)
        nc.sync.dma_start(x2k[:], d_x2k)

        logits = p_const.tile([128, 4, 4, 256], FP32, tag="logits")
        vT = p_const.tile([128, 8, 64], BF16, tag="vT")      # squash output
        sT_sb = p_const.tile([128, 8, 64], FP32, tag="sT_sb")
        e_t = p_const.tile([128, 16, 256], BF16, tag="e_t")  # exp(logits)
        c_t = p_const.tile([128, 16, 256], BF16, tag="c_t")  # softmax probs
        zrow = p_const.tile([128, 256], FP32, tag="zrow")    # per-half sum + recip

        # ================================================ pass 0
        # s0[b, jd] accumulated over 32 km-tiles; moving = streamed Wt tile.
        ps_s0 = p_ps_g.tile([64, 1024], FP32, tag="ps")
        for t2a in range(16):
            wt_t = p_wstream.tile([128, 2, 1024], BF16, tag="wt_s")
            nc.sync.dma_start(wt_t[:], d_Wt[:, 2 * t2a:2 * t2a + 2, :])
            for tl in range(2):
                t2 = 2 * t2a + tl
                for half in range(2):
                    nc.tensor.matmul(
                        ps_s0[:, 512 * half:512 * half + 512],
                        xT[:, t2, :],                  # stationary [128, 64]
                        wt_t[:, tl, 512 * half:512 * half + 512],
                        start=(t2 == 0), stop=(t2 == 31),
                    )
        s0_sb = p_small.tile([64, 1024], FP32, tag="s0_sb")
        nc.vector.tensor_copy(s0_sb[:], ps_s0[:])
        # PE-transpose 8 blocks [64, 128] -> s0T psum [128, 8, 64]
        ps_s0T = p_ps_g.tile([128, 8, 64], FP32, tag="ps")
        for m in range(8):
            nc.tensor.transpose(ps_s0T[:, m, :], s0_sb[:, 128 * m:128 * m + 128],
                                ident[0:64, 0:64])
        nc.vector.tensor_copy(sT_sb[:], ps_s0T[:])

        def allreduce_sT():
            nc.sync.dma_start(cc_in[:], sT_sb[:])
            nc.gpsimd.collective_compute(
                "AllReduce", mybir.AluOpType.add,
                replica_groups=[core_ids],
                ins=[cc_in[:]], outs=[cc_out[:]],
            )
            nc.sync.dma_start(sT_sb[:], cc_out[:])

        def squash(out_bf16, out_fp32=None):
            """sT_sb [128,8,64] -> vT (bf16) and optionally fp32 copy.

            scale = s2/(1+s2) / sqrt(s2+eps); sqrt via ACT + one Newton step
            (ACT Sqrt table has a loose precision budget), divides via DVE
            bit-exact reciprocal.
            """
            sq = p_small.tile([128, 8, 64], FP32, tag="sq")
            nc.vector.tensor_tensor(sq[:], sT_sb[:], sT_sb[:],
                                    op=mybir.AluOpType.mult)
            ps_s2 = p_ps_g.tile([4, 8, 64], FP32, tag="ps")
            for slot in range(8):
                nc.tensor.matmul(ps_s2[:, slot, :], ones4, sq[:, slot, :],
                                 start=True, stop=True)
            s2 = p_small.tile([4, 8, 64], FP32, tag="s2")
            nc.vector.tensor_copy(s2[:], ps_s2[:])
            t = p_small.tile([4, 8, 64], FP32, tag="t")
            nc.vector.tensor_scalar(t[:], s2[:], EPS, None,
                                    op0=mybir.AluOpType.add)
            y = p_small.tile([4, 8, 64], FP32, tag="y")
            nc.scalar.sqrt(y[:], t[:])
            # Newton for sqrt: y' = 0.5*(y + t/y)
            ry = p_small.tile([4, 8, 64], FP32, tag="ry")
            nc.vector.reciprocal(ry[:], y[:])
            nc.vector.tensor_tensor(ry[:], ry[:], t[:], op=mybir.AluOpType.mult)
            nc.vector.tensor_tensor(y[:], y[:], ry[:], op=mybir.AluOpType.add)
            nc.vector.tensor_scalar(y[:], y[:], 0.5, None,
                                    op0=mybir.AluOpType.mult)
            # den = (1+s2)*y ; scale = s2 * recip(den)
            den = p_small.tile([4, 8, 64], FP32, tag="den")
            nc.vector.tensor_scalar(den[:], s2[:], 1.0, None,
                                    op0=mybir.AluOpType.add)
            nc.vector.tensor_tensor(den[:], den[:], y[:], op=mybir.AluOpType.mult)
            nc.vector.reciprocal(den[:], den[:])
            scl = p_small.tile([4, 8, 64], FP32, tag="scl")
            nc.vector.tensor_tensor(scl[:], den[:], s2[:], op=mybir.AluOpType.mult)
            # replicate over d: ps_rep [128, 8, 64] = repM^T . scl
            ps_rep = p_ps_g.tile([128, 8, 64], FP32, tag="ps")
            for slot in range(8):
                nc.tensor.matmul(ps_rep[:, slot, :], repM, scl[:, slot, :],
                                 start=True, stop=True)
            nc.vector.tensor_tensor(out_bf16[:], sT_sb[:], ps_rep[:],
                                    op=mybir.AluOpType.mult)
            if out_fp32 is not None:
                nc.vector.tensor_tensor(out_fp32[:], sT_sb[:], ps_rep[:],
                                        op=mybir.AluOpType.mult)

        allreduce_sT()
        # fold the 1/Nc uniform-c scale: xT was pre-scaled on host.
        squash(vT)

        # ================================================ passes 1, 2
        for pas in range(2):
            # ---------------- (A): G = Wd . vT ; logits += sum_k x2a * G
            for ga in range(4):
                for cha in range(4):
                    wd_t = p_wstream.tile([128, 2, 1024], BF16, tag="wd_s")
                    nc.sync.dma_start(wd_t[:],
                                      d_Wd[:, 2 * ga:2 * ga + 2,
                                           1024 * cha:1024 * cha + 1024])
                    for chl in range(2):
                        ch = 2 * cha + chl
                        ps_G = p_ps_g.tile([128, 4, 512], FP32, tag="ps")
                        for r in range(4):
                            for c2 in range(2):
                                nc.tensor.matmul(
                                    ps_G[64 * c2:64 * c2 + 64, r, :],
                                    vT[32 * r:32 * r + 32, 2 * ga + c2, :],
                                    wd_t[32 * r:32 * r + 32, c2,
                                         512 * chl:512 * chl + 512],
                                    start=True, stop=True,
                                    tile_position=(32 * r, 64 * c2),
                                )
                        gev = p_gevac.tile([128, 4, 512], BF16, tag="gev")
                        nc.scalar.copy(gev[:], ps_G[:])
                        prod = p_prod.tile([128, 4, 512], BF16, tag="prod")
                        x2sl = x2a[:, 512 * ch:512 * ch + 512]
                        nc.vector.tensor_tensor(
                            prod[:], gev[:],
                            x2sl.unsqueeze(1).broadcast_to((128, 4, 512)),
                            op=mybir.AluOpType.mult)
                        # bf16 out keeps the reduce in the DVE 2x perf mode
                        # (fp32 out would force 1x); internal accum is fp32.
                        red = p_red.tile([128, 4, 32], BF16, tag="red")
                        with nc.allow_low_precision("bf16 logit increment"):
                            nc.vector.reduce_sum(
                                red[:],
                                prod[:].rearrange("p r (i k) -> p r i k", k=16),
                                axis=mybir.AxisListType.X)
                        lsl = logits[:, ga, :, 32 * ch:32 * ch + 32]
                        if pas == 0:
                            nc.vector.tensor_copy(lsl, red[:])
                        else:
                            nc.vector.tensor_tensor(lsl, lsl, red[:],
                                                    op=mybir.AluOpType.add)
            # ---------------- softmax over j (split-j layout)
            nc.scalar.activation(e_t[:].rearrange("p a b -> p (a b)"),
                                 logits[:].rearrange("p g r i -> p (g r i)"),
                                 mybir.ActivationFunctionType.Exp)
            # Zh = sum over jj (outer axis): AP [128, i, jj] reduce X
            nc.vector.reduce_sum(zrow[:],
                                 e_t[:].rearrange("p jj i -> p i jj"),
                                 axis=mybir.AxisListType.X)
            # cross-half add: copy upper half partitions down, add, recip,
            # then copy recip back up.
            ztmp = p_small.tile([64, 256], FP32, tag="ztmp")
            nc.sync.dma_start(ztmp[:], zrow[64:128, :])
            nc.vector.tensor_tensor(zrow[0:64, :], zrow[0:64, :], ztmp[:],
                                    op=mybir.AluOpType.add)
            nc.vector.reciprocal(zrow[0:64, :], zrow[0:64, :])
            nc.sync.dma_start(zrow[64:128, :], zrow[0:64, :])
            nc.vector.tensor_tensor(
                c_t[:], e_t[:],
                zrow[:].unsqueeze(1).broadcast_to((128, 16, 256)),
                op=mybir.AluOpType.mult)
            # ---------------- (B): xc2 -> DMA-transpose -> PE contraction
            last = (pas == 1)
            ps_sT = p_ps_g.tile([128, 8, 64], FP32, tag="ps")
            for m in range(4):
                xcT_bufs = []
                for jq in range(4):
                    jj = 4 * m + jq
                    xc2 = p_xc.tile([128, 4096], BF16, tag="xc2")
                    # c broadcast over k (outer step-0), i inner step-1
                    cap = c_t[:, jj, :].unsqueeze(1).broadcast_to((128, 16, 256))
                    nc.vector.tensor_tensor(
                        xc2[:].rearrange("p (k i) -> p k i", i=256),
                        x2k[:].rearrange("p (k i) -> p k i", i=256),
                        cap, op=mybir.AluOpType.mult)
                    # one block-wise transpose: xcT[p, t2, f] = xc2[f, 128*t2+p]
                    xcT = p_xcT.tile([128, 32, 128], BF16, tag="xcT")
                    nc.sync.dma_start(xcT[:], xc2[:], transpose=True)
                    xcT_bufs.append(xcT)
                # stationary slab for all 8 j's of this m, all t2
                wtm = p_wm.tile([128, 32, 256], BF16, tag="wtm")
                nc.sync.dma_start(wtm[:], d_Wt[:, :, 256 * m:256 * m + 256])
                # t2 INNERMOST: each accumulation group completes before the
                # next starts (start=True clears has_written bank-wide).
                for gq in (2 * m, 2 * m + 1):
                    c2 = gq % 2
                    for q in range(4):
                        j = 4 * gq + q           # j%4 == q, jj = 4*m + q
                        jl = j - 8 * m
                        for t2 in range(32):
                            nc.tensor.matmul(
                                ps_sT[32 * q:32 * q + 32, gq, :],
                                wtm[:, t2, 32 * jl:32 * jl + 32],
                                xcT_bufs[q][:, t2, 64 * c2:64 * c2 + 64],
                                start=(t2 == 0), stop=(t2 == 31),
                                tile_position=(0, 32 * q),
                                skip_group_check=True,
                            )
            nc.vector.tensor_copy(sT_sb[:], ps_sT[:])
            allreduce_sT()
            if not last:
                squash(vT)
            else:
                vfin = p_small.tile([128, 8, 64], FP32, tag="vfin")
                squash(vT, out_fp32=vfin)
                nc.sync.dma_start(d_out, vfin[:])

    nc.compile()
    return nc


def kernel(x, W):
    x = np.asarray(x, dtype=np.float32)
    W = np.asarray(W, dtype=np.float32)
    in_maps = [_host_prep_core(x, W, c) for c in range(NCORES)]

    nc = build_program()
    res = run_bass_kernel_spmd(nc, in_maps, list(range(NCORES)))
    vT = res.results[0]["out"]  # [128, 8, 64]

    v = np.empty((B, Nc, Dc), np.float32)
    for j in range(Nc):
        v[:, j, :] = vT[32 * (j % 4):32 * (j % 4) + 32, j // 4, :].T
    return v


if __name__ == "__main__":
    rng = np.random.default_rng(0)
    x = rng.standard_normal((B, In, Din), dtype=np.float32)
    W = (rng.standard_normal((Nc, In, Dc, Din), dtype=np.float32) * 0.05)
    out = kernel(x, W)
    print("kernel ran; out shape", out.shape, "mean", float(np.abs(out).mean()))
